# revision 1
# baseline (speedup 1.0000x reference)
"""TRN2 Bass kernel for nn_Cvx_KnapsackNet (MLP + 200-iter ADMM projection QP).

Math: the reference ADMM iteration collapses algebraically. With
P' = (I - A^T (A A^T)^{-1} A)/(1+rho), c = b @ (A A^T)^{-1} A, and
state q_k = x_k + u_{k-1}:
    t_k     = w + |q_k|          (t_0 = w)
    x_k     = t_k @ P' + c
    q_{k+1} = x_k + min(q_k, 0)
c is folded into the matmul via an extra "ones" row (row 1030 of the
padded state is held at 1; row 1030 of P' holds c).

Acceleration: the first NPR iterations run *over-relaxed* ADMM with
alpha=2 (Peaceman-Rachford). For alpha=2 the relaxed update
    q' = q + alpha*(x - relu(q))
collapses to q' = 2x - |q|, so with Pt = 2*P' (and 2c in the bias row)
the whole update is one tensor-tensor subtract against the matmul
output. PR roughly halves the iterations needed; NFIN plain ADMM
iterations follow to settle the active set (the plain iterations feed
the matmul t/2 -- using half-scaled |q| and w -- so the same doubled
Pt matrix yields the unscaled x). Converges to ~2e-3 relative error
(bf16 floor) in 8 PR + 2 plain iterations vs 16 plain.

Everything runs in bf16 on the PE (fp32 PSUM accumulation): bf16
matmuls are 4x faster than fp32 and halve the dominant W2 HBM stream.
Measured end-to-end error ~3e-3 vs the 2e-2 target.

Schedule: HBM-bound MLP, tensor-bound ADMM.
- W2 host-packed partition-major; 5 chunks, each loaded as 2 DMAs so
  compute trails the stream by half a chunk; triple buffered.
- Cost-layer (W3) matmuls interleave into layer-2's DMA shadow,
  accumulating into SBUF so PSUM stays within 8 banks.
- P' (bf16, j-major) streams in 3 DMAs anchored under the last W2
  chunk's compute, just in time for ADMM iteration 0.
- ADMM elementwise per tile: TT subtract (PR) / fp32 STT (plain),
  ScalarE Abs -> bf16, all-bf16 TT add (2x DVE mode).

Sharding: pure data parallel, batch 1024 -> 128 rows per NeuronCore.
On-chip layout is transposed ([n2p=1152 rows, 128 batch cols], 9 tiles
of 128 partitions) so the matmul contraction runs over partitions.
"""
import sys
sys.path.insert(0, '/opt/trn_rl_repo')
import os
import numpy as np

B, C, H, R, K = 1024, 32, 3200, 500, 30
RHO = 1.0
N1 = K + R              # 530
N2 = R + K + R          # 1030
N2P = 1152              # 9 * 128
NT = N2P // 128         # 9 state tiles
BIAS_ROW = N2           # 1030
NCORES = 8
BL = B // NCORES        # 128 batch rows per core
HT = H // 128           # 25 hidden tiles
NPR = int(os.environ.get("KNAP_PR", "4"))     # Peaceman-Rachford iters
NFIN = int(os.environ.get("KNAP_FIN", "2"))   # plain ADMM finishers
MC_W = 5                # m-tiles per W2 chunk
N_MC = HT // MC_W       # 5 chunks
CT = 512 // 128         # 4 cost tiles (500 padded to 512)

_CACHE = {}


def _host_precompute(W1, b1, W2, b2, W3, b3, weights_mat, capacities):
    """float64 host math -> packed bf16/fp32 device constants."""
    import ml_dtypes
    bf = ml_dtypes.bfloat16
    wm = weights_mat.astype(np.float64)
    cap = capacities.astype(np.float64)
    A = np.zeros((N1, N2), np.float64)
    A[:K, :R] = wm
    A[:K, R:R + K] = np.eye(K)
    A[K:, :R] = np.eye(R)
    A[K:, R + K:] = np.eye(R)
    b = np.concatenate([cap, np.ones(R)])
    M = np.linalg.inv(A @ A.T)
    P = (np.eye(N2) - A.T @ M @ A) / (1.0 + RHO)
    c = b @ M @ A
    Pbig = np.zeros((N2P, N2P), np.float32)
    Pbig[:N2, :N2] = 2.0 * P.astype(np.float32)      # Pt = 2 P'
    Pbig[BIAS_ROW, :N2] = 2.0 * c.astype(np.float32)
    # j-major blocked: PbigPM[p, (j*NT+k)*128 + f] = Pbig[k*128+p, j*128+f]
    PbigPM = np.ascontiguousarray(
        Pbig.reshape(NT, 128, NT, 128).transpose(1, 2, 0, 3).reshape(128, NT * NT * 128))
    PbigBF = PbigPM.astype(bf)

    W3p = np.zeros((512, H), np.float32)
    W3p[:R] = W3
    # w3PM[p, k*512 + f] = W3p.T[k*128+p, f]
    w3PM = np.ascontiguousarray(
        W3p.T.reshape(HT, 128, 512).transpose(1, 0, 2).reshape(128, HT * 512)).astype(bf)

    b1R = np.ascontiguousarray(b1.reshape(HT, 128).T)       # [128, 25]
    b2R = np.ascontiguousarray(b2.reshape(HT, 128).T)       # [128, 25]
    b3p = np.zeros(512, np.float32)
    b3p[:R] = b3
    b3R = np.ascontiguousarray(b3p.reshape(CT, 128).T)      # [128, 4]
    b3Rh = 0.5 * b3R                                        # [128, 4]
    # padding tiles 4..8 of w (zeros; bias-row 1030 -> tile 8, partition 6 = 1)
    wpad = np.zeros((128, (NT - CT) * 128), np.float32)
    wpad[BIAS_ROW - 8 * 128, (8 - CT) * 128:(9 - CT) * 128] = 1.0

    small = np.concatenate([b1R, b2R, b3R, b3Rh, wpad], axis=1).astype(np.float32)
    # w2PM[p, (mc*HT + k)*(MC_W*128) + f] = W2.T[k*128+p, mc*MC_W*128+f]
    W2T = np.ascontiguousarray(W2.T)                        # [3200, 3200] (in, out)
    w2PM = np.ascontiguousarray(
        W2T.reshape(HT, 128, N_MC, MC_W * 128).transpose(1, 2, 0, 3)
           .reshape(128, H * H // 128)).astype(bf)          # [128, 80000]
    W1T = np.ascontiguousarray(W1.T).astype(bf)             # [32, 3200]
    return small, PbigBF, w3PM, W1T, w2PM


def _build_nc():
    import concourse.bacc as bacc
    import concourse.mybir as mybir
    from concourse import tile
    from concourse.tile_rust import add_dep_helper

    f32 = mybir.dt.float32
    bf16 = mybir.dt.bfloat16
    SMALL_W = HT + HT + CT + CT + (NT - CT) * 128
    OFF_B1 = 0
    OFF_B2 = OFF_B1 + HT
    OFF_B3 = OFF_B2 + HT
    OFF_B3H = OFF_B3 + CT
    OFF_WP = OFF_B3H + CT
    MCW = MC_W * 128        # 640 cols per W2 chunk
    W2CH = HT * MCW         # 16000 elems per partition per chunk
    W2SPLIT = [0, 9 * MCW, 17 * MCW, W2CH]   # chunk DMA split in thirds
    PBF_W = NT * NT * 128   # 10368
    PBF_CH = [(0, 3), (3, 3), (6, 3)]   # j-ranges per pbf DMA
    TOTAL = NPR + NFIN
    assert NPR >= 1 and NFIN >= 1

    nc = bacc.Bacc("TRN2", target_bir_lowering=False, debug=False, num_devices=NCORES)
    small_d = nc.dram_tensor("small_d", [128, SMALL_W], f32, kind="ExternalInput").ap()
    pbf_d = nc.dram_tensor("pbf_d", [128, PBF_W], bf16, kind="ExternalInput").ap()
    w3_d = nc.dram_tensor("w3_d", [128, HT * 512], bf16, kind="ExternalInput").ap()
    dw_d = nc.dram_tensor("dw_d", [C, BL + H], bf16, kind="ExternalInput").ap()
    w2_d = nc.dram_tensor("w2_d", [128, N_MC * W2CH], bf16, kind="ExternalInput").ap()
    out_d = nc.dram_tensor("out_d", [128, N2P], f32, kind="ExternalOutput").ap()

    Act = mybir.ActivationFunctionType
    Alu = mybir.AluOpType

    with tile.TileContext(nc) as tc:
        with tc.tile_pool(name="sb", bufs=1) as sb, \
             tc.tile_pool(name="wst", bufs=3) as wst, \
             tc.tile_pool(name="mlp", bufs=1) as mlp, \
             tc.tile_pool(name="ps", bufs=8, space="PSUM") as pspool:
            dw = mlp.tile([C, BL + H], bf16)
            nc.sync.dma_start(out=dw[:], in_=dw_d[:])
            sm = sb.tile([128, SMALL_W], f32)
            nc.sync.dma_start(out=sm[:], in_=small_d[:])
            w3sb = sb.tile([128, HT * 512], bf16)
            nc.sync.dma_start(out=w3sb[:], in_=w3_d[:])
            pbf = sb.tile([128, PBF_W], bf16)

            b1R = sm[:, OFF_B1:OFF_B1 + HT]
            b2R = sm[:, OFF_B2:OFF_B2 + HT]
            b3R = sm[:, OFF_B3:OFF_B3 + CT]
            b3Rh = sm[:, OFF_B3H:OFF_B3H + CT]
            dT = dw[:, 0:BL]
            w1T = dw[:, BL:BL + H]

            h1 = mlp.tile([128, HT * 128], bf16)  # h1T tiles: [p, m*128+b]
            h2 = mlp.tile([128, HT * 128], bf16)
            w_acc = mlp.tile([128, 512], f32)     # cost-layer SBUF accumulator
            wb_sb = sb.tile([128, N2P], bf16)     # w      (PR iterations)
            wh_sb = sb.tile([128, N2P], bf16)     # w / 2  (plain iterations)
            q_sb = sb.tile([128, N2P], f32)
            a_sb = sb.tile([128, N2P], bf16)
            tb_bufs = [sb.tile([128, N2P], bf16, name=f"tb{i}") for i in range(3)]
            out_sb = sb.tile([128, N2P], f32)

            nc.vector.memset(a_sb[:], 0.0)
            # pad tiles 4..8 of w (zeros + bias-one row): full and half copies
            nc.vector.tensor_copy(wb_sb[:, CT * 128:],
                                  sm[:, OFF_WP:OFF_WP + (NT - CT) * 128])
            nc.scalar.activation(wh_sb[:, CT * 128:],
                                 sm[:, OFF_WP:OFF_WP + (NT - CT) * 128],
                                 Act.Copy, scale=0.5)

            # ---- MLP layer 1: h1T[m] = prelu(W1T[:,m].T @ dT + b1, 0.1) ----
            for m in range(HT):
                ps_t = pspool.tile([128, 128], f32, tag="ps", name="ps_t")
                nc.tensor.matmul(ps_t[:], w1T[:, m * 128:(m + 1) * 128], dT,
                                 start=True, stop=True)
                nc.scalar.activation(h1[:, m * 128:(m + 1) * 128], ps_t[:],
                                     Act.Prelu, bias=b1R[:, m:m + 1], alpha=0.1)

            # ---- MLP layer 2 + interleaved cost layer ----
            chunk_mark = {}

            def emit_cost(mc):
                ps_c = [pspool.tile([128, 128], f32, tag="ps", name="ps_t")
                        for _ in range(CT)]
                for ki in range(MC_W):
                    k = mc * MC_W + ki
                    for m in range(CT):
                        nc.tensor.matmul(ps_c[m][:],
                                         w3sb[:, k * 512 + m * 128:
                                                 k * 512 + (m + 1) * 128],
                                         h2[:, k * 128:(k + 1) * 128],
                                         start=(ki == 0), stop=(ki == MC_W - 1))
                for m in range(CT):
                    mm = slice(m * 128, (m + 1) * 128)
                    if mc == 0:
                        nc.scalar.activation(w_acc[:, mm], ps_c[m][:], Act.Copy)
                    else:
                        nc.vector.tensor_tensor(out=w_acc[:, mm], in0=w_acc[:, mm],
                                                in1=ps_c[m][:], op=Alu.add)
                if mc == N_MC - 1:
                    for m in range(CT):
                        mm = slice(m * 128, (m + 1) * 128)
                        nc.scalar.activation(wb_sb[:, mm], w_acc[:, mm],
                                             Act.Identity, bias=b3R[:, m:m + 1])
                    for m in range(CT):
                        mm = slice(m * 128, (m + 1) * 128)
                        nc.scalar.activation(wh_sb[:, mm], w_acc[:, mm],
                                             Act.Identity, bias=b3Rh[:, m:m + 1],
                                             scale=0.5)

            for mc in range(N_MC):
                w2blk = wst.tile([128, W2CH], bf16, name="w2blk")
                for s0, s1 in zip(W2SPLIT[:-1], W2SPLIT[1:]):
                    nc.sync.dma_start(out=w2blk[:, s0:s1],
                                      in_=w2_d[:, mc * W2CH + s0:mc * W2CH + s1])
                if mc == N_MC - 1:
                    # P' queues right behind the last W2 chunk on the DMA
                    # engines; program order keeps the stream gap-free.
                    for j0, jn in PBF_CH:
                        nc.sync.dma_start(
                            out=pbf[:, j0 * NT * 128:(j0 + jn) * NT * 128],
                            in_=pbf_d[:, j0 * NT * 128:(j0 + jn) * NT * 128])
                if mc >= 1:
                    # cost matmuls for the previous chunk run in this chunk's
                    # DMA-wait gap and free their PSUM banks early
                    emit_cost(mc - 1)
                ps_list = [pspool.tile([128, 128], f32, tag="ps", name="ps_t")
                           for _ in range(MC_W)]
                last_mc = (mc == N_MC - 1)
                # For the last chunk, the final DMA third runs mi-grouped so
                # each h2 tile (and its cost matmuls) completes as early as
                # possible, shortening the serial tail into ADMM iter 0.
                KSPLIT = 17 if last_mc else HT
                for k in range(KSPLIT):
                    for mi in range(MC_W):
                        mm = nc.tensor.matmul(ps_list[mi][:],
                                         w2blk[:, k * MCW + mi * 128:
                                                  k * MCW + (mi + 1) * 128],
                                         h1[:, k * 128:(k + 1) * 128],
                                         start=(k == 0), stop=(k == HT - 1))
                        if k == 0 and mi == 0:
                            chunk_mark[mc] = mm.ins
                if not last_mc:
                    for mi in range(MC_W):
                        m = mc * MC_W + mi
                        nc.scalar.activation(h2[:, m * 128:(m + 1) * 128],
                                             ps_list[mi][:], Act.Prelu,
                                             bias=b2R[:, m:m + 1], alpha=0.1)
            # ---- last chunk tail: mi-grouped third + pipelined cost ----
            mc = N_MC - 1
            ps_c = [pspool.tile([128, 128], f32, tag="ps", name="ps_t")
                    for _ in range(CT)]

            def cost_piece(ki):
                k = mc * MC_W + ki
                for m in range(CT):
                    nc.tensor.matmul(ps_c[m][:],
                                     w3sb[:, k * 512 + m * 128:
                                             k * 512 + (m + 1) * 128],
                                     h2[:, k * 128:(k + 1) * 128],
                                     start=(ki == 0), stop=(ki == MC_W - 1))

            for mi in range(MC_W):
                for k in range(17, HT):
                    nc.tensor.matmul(ps_list[mi][:],
                                     w2blk[:, k * MCW + mi * 128:
                                              k * MCW + (mi + 1) * 128],
                                     h1[:, k * 128:(k + 1) * 128],
                                     start=False, stop=(k == HT - 1))
                m = mc * MC_W + mi
                nc.scalar.activation(h2[:, m * 128:(m + 1) * 128], ps_list[mi][:],
                                     Act.Prelu, bias=b2R[:, m:m + 1], alpha=0.1)
                if mi >= 1:
                    cost_piece(mi - 1)
            cost_piece(MC_W - 1)
            for m in range(CT):
                mm = slice(m * 128, (m + 1) * 128)
                nc.vector.tensor_tensor(out=w_acc[:, mm], in0=w_acc[:, mm],
                                        in1=ps_c[m][:], op=Alu.add)
            for m in range(CT):
                mm = slice(m * 128, (m + 1) * 128)
                nc.scalar.activation(wb_sb[:, mm], w_acc[:, mm],
                                     Act.Identity, bias=b3R[:, m:m + 1])
            for m in range(CT):
                mm = slice(m * 128, (m + 1) * 128)
                nc.scalar.activation(wh_sb[:, mm], w_acc[:, mm],
                                     Act.Identity, bias=b3Rh[:, m:m + 1],
                                     scale=0.5)

            # ---- ADMM: NPR Peaceman-Rachford iters + NFIN plain iters ----
            for it in range(TOTAL):
                last = (it == TOTAL - 1)
                pr = it < NPR
                half_out = (it >= NPR - 1)      # feed t/2 to plain iterations
                cur = wb_sb if it == 0 else tb_bufs[(it - 1) % 3]
                for j in range(NT):
                    ps_t = pspool.tile([128, 128], f32, tag="ps", name="ps_t")
                    for k in range(NT):
                        nc.tensor.matmul(ps_t[:],
                                         pbf[:, (j * NT + k) * 128:(j * NT + k + 1) * 128],
                                         cur[:, k * 128:(k + 1) * 128],
                                         start=(k == 0), stop=(k == NT - 1))
                    jj = slice(j * 128, (j + 1) * 128)
                    if last:
                        nc.scalar.activation(out_sb[:, jj], ps_t[:], Act.Copy)
                        if j % 3 == 2:
                            oo = slice((j - 2) * 128, (j + 1) * 128)
                            nc.sync.dma_start(out=out_d[:, oo], in_=out_sb[:, oo])
                        continue
                    if pr:
                        # q' = 2x - |q|  (Pt already holds the factor 2)
                        nc.vector.tensor_tensor(out=q_sb[:, jj], in0=ps_t[:],
                                                in1=a_sb[:, jj], op=Alu.subtract)
                    else:
                        # q' = min(q, 0) + x
                        nc.vector.scalar_tensor_tensor(
                            out=q_sb[:, jj], in0=q_sb[:, jj], scalar=0.0,
                            in1=ps_t[:], op0=Alu.min, op1=Alu.add)
                    nc.scalar.activation(a_sb[:, jj], q_sb[:, jj], Act.Abs,
                                         scale=0.5 if half_out else 1.0)
                    # t-build on GpSimd (SBUF-only bf16 add) keeps VectorE at
                    # one op/tile so the iteration sits at the tensor floor
                    nc.gpsimd.tensor_tensor(out=tb_bufs[it % 3][:, jj],
                                            in0=a_sb[:, jj],
                                            in1=(wh_sb if half_out else wb_sb)[:, jj],
                                            op=Alu.add)

    nc.compile()
    return nc


def kernel(d, W1, b1, W2, b2, W3, b3, weights_mat, capacities):
    import ml_dtypes
    from concourse.bass_utils import run_bass_kernel_spmd

    d = np.asarray(d, np.float32)
    small, PbigBF, w3PM, W1T, w2PM = _host_precompute(
        np.asarray(W1, np.float32), np.asarray(b1, np.float32),
        np.asarray(W2, np.float32), np.asarray(b2, np.float32),
        np.asarray(W3, np.float32), np.asarray(b3, np.float32),
        np.asarray(weights_mat, np.float32), np.asarray(capacities, np.float32))

    if "nc" not in _CACHE:
        _CACHE["nc"] = _build_nc()
    nc = _CACHE["nc"]

    in_maps = []
    for i in range(NCORES):
        dTc = np.ascontiguousarray(d[i * BL:(i + 1) * BL].T)      # [32, 128]
        dwc = np.concatenate([dTc.astype(ml_dtypes.bfloat16), W1T], axis=1)
        in_maps.append({"small_d": small, "pbf_d": PbigBF,
                        "w3_d": w3PM, "dw_d": dwc, "w2_d": w2PM})

    trace = bool(int(os.environ.get("KNAP_TRACE", "0")))
    res = run_bass_kernel_spmd(nc, in_maps, core_ids=list(range(NCORES)),
                               trace=trace)
    if trace:
        _CACHE["exec_time_ns"] = res.exec_time_ns
        _CACHE["trace"] = res.instructions_and_trace

    out = np.empty((B, N2), np.float32)
    for i in range(NCORES):
        arr = res.results[i]["out_d"]                              # [128, 1152]
        xc = arr.reshape(128, NT, 128).transpose(2, 1, 0).reshape(BL, N2P)
        out[i * BL:(i + 1) * BL] = xc[:, :N2]
    return out



# revision 10
# speedup vs baseline: 1.0108x; 1.0108x over previous
"""TRN2 Bass kernel for nn_Cvx_KnapsackNet (MLP + ADMM projection QP).

Math: with N = A^T M A (M = inv(A A^T), rank 530) and r := w - Nw + 2c
computed once, the alpha=2 (Peaceman-Rachford) ADMM iteration collapses to
    q' = r - N|q|            (PR iters;  q1 = r)
    q' = (q + r - N|q|)/2    (plain finisher iters)
    x  = (r + |q| - N|q|)/2  (final output)
The N-apply is factored through the 530-dim dual space:
    S = A a  (S_top = wm@a_r + a_k, 4 mm; S_bot = a_r + a_s, 1 vector op)
    U_top = M_K S (5 mm), U_bot = M_R S (20 mm), V_r = G_r S (20 mm)
with G_r = wm^T M_K + M_R, so N a = [V_r; U_top; U_bot] in primal blocks
(r=500, k=30, s=500). ~49 matmuls/iter vs 81 dense. The -2c offsets ride a
constant "ones" contraction row in the packs (enabled only for the r-pass).
b1 rides an ones-row in dT; b3 rides a rank-1 matmul into the cost PSUM.

Everything fp16 (fp32 PSUM): same PE/DMA cost as bf16, 8x less rounding
noise. KNAP_W2FP8=1 switches the dominant W2 stream to fp8e4 (halves its
HBM traffic; prelu rescales by 1/64).

Schedule: DMA prefix reordered so W2 chunk 0 streams immediately; w3
pieces and ADMM packs ride just-in-time between W2 chunks. Cost layer
accumulates into a single PSUM bank across all chunks. ADMM elementwise
is spread: wide [128,512] ops on vector/scalar, k-block on gpsimd.

Sharding: pure data parallel, batch 1024 -> 128 rows per core, on-chip
layout transposed [feature partitions, batch cols].
"""
import sys
sys.path.insert(0, '/opt/trn_rl_repo')
import os
import numpy as np

B, C, H, R, K = 1024, 32, 3200, 500, 30
N1 = K + R              # 530
N2 = R + K + R          # 1030
NCORES = 8
BL = B // NCORES        # 128 batch rows per core
HT = H // 128           # 25 hidden tiles
MC_W = 5                # m-tiles per W2 chunk
N_MC = HT // MC_W       # 5 chunks
MCW = MC_W * 128        # 640
W2CH = HT * MCW         # 16000 elems/partition/chunk
W2SPLIT = [0, 9 * MCW, 17 * MCW, W2CH]
CT = 4                  # cost tiles (500 -> 512)
NPR = int(os.environ.get("KNAP_PR", "4"))
NFIN = int(os.environ.get("KNAP_FIN", "2"))
TOTAL = NPR + NFIN
W2FP8 = bool(int(os.environ.get("KNAP_W2FP8", "0")))
W2SCALE = 64.0
# state layout [128, 1152]: r cols 0:512, k cols 512:640 (parts 0:30), s 640:1152
OK_, OS_, SW = 512, 640, 1152
# packs column layout (fp16)
PK_WMT = 0                     # 4 k-tiles x 30
PK_MK = PK_WMT + 4 * 30        # 5 x 30
PK_MR = PK_MK + 5 * 30         # (m*5+t) x 128, m<4 t<5
PK_GR = PK_MR + 20 * 128
PK_B3 = PK_GR + 20 * 128       # 512 (partition 0 only)
PACKW = PK_B3 + 512

_CACHE = {}


def _host_precompute(W1, b1, W2, b2, W3, b3, weights_mat, capacities):
    """float64 host math -> packed fp16/fp32 device constants."""
    import ml_dtypes
    f16 = np.float16
    wm = weights_mat.astype(np.float64)
    cap = capacities.astype(np.float64)
    A = np.zeros((N1, N2), np.float64)
    A[:K, :R] = wm
    A[:K, R:R + K] = np.eye(K)
    A[K:, :R] = np.eye(R)
    A[K:, R + K:] = np.eye(R)
    b = np.concatenate([cap, np.ones(R)])
    M = np.linalg.inv(A @ A.T)
    c = b @ M @ A                            # [N2]
    c_r, c_k, c_s = c[:R], c[R:R + K], c[R + K:]

    # dual padded index map [640] -> 0..529 (K block 0:30 at tile0, R at 1..4)
    didx = np.full(640, -1, np.int64)
    didx[0:K] = np.arange(K)
    for t in range(1, 5):
        base = (t - 1) * 128
        n = min(128, R - base)
        didx[t * 128:t * 128 + n] = K + base + np.arange(n)
    valid = didx >= 0
    Mp = np.zeros((640, N1))
    Mp[valid] = M[:, didx[valid]].T          # Mp[dp, j] = M[j, didx[dp]]
    Gfull = np.zeros((512, N1))
    Gfull[:R] = wm.T @ M[:K] + M[K:]         # G_r [500, 530]
    Gp = np.zeros((640, 512))
    Gp[valid] = Gfull[:, didx[valid]].T

    MKmat = Mp[:, :K].copy()                 # [640, 30]
    MRmat = np.zeros((640, 512))
    MRmat[:, :R] = Mp[:, K:]
    GRmat = Gp                               # [640, 512]
    # -2c offsets on the ones contraction row (tile 0, partition 30)
    MKmat[32, :] = -2.0 * c_k
    MRmat[32, :R] = -2.0 * c_s
    GRmat[32, :R] = -2.0 * c_r

    packs = np.zeros((128, PACKW), np.float32)
    wmT = np.zeros((512, K))
    wmT[:R] = wm.T
    for j in range(4):
        packs[:, PK_WMT + j * 30:PK_WMT + (j + 1) * 30] = wmT[j * 128:(j + 1) * 128]
    for t in range(5):
        packs[:, PK_MK + t * 30:PK_MK + (t + 1) * 30] = MKmat[t * 128:(t + 1) * 128]
        for m in range(4):
            packs[:, PK_MR + (m * 5 + t) * 128:PK_MR + (m * 5 + t + 1) * 128] = \
                MRmat[t * 128:(t + 1) * 128, m * 128:(m + 1) * 128]
            packs[:, PK_GR + (m * 5 + t) * 128:PK_GR + (m * 5 + t + 1) * 128] = \
                GRmat[t * 128:(t + 1) * 128, m * 128:(m + 1) * 128]
    b3p = np.zeros(512)
    b3p[:R] = b3
    packs[0, PK_B3:PK_B3 + 512] = b3p
    packsF = packs.astype(f16)

    b2R = np.ascontiguousarray(b2.reshape(HT, 128).T).astype(np.float32)  # [128,25]

    W3p = np.zeros((512, H), np.float32)
    W3p[:R] = W3
    w3PM = np.ascontiguousarray(
        W3p.T.reshape(HT, 128, 512).transpose(1, 0, 2).reshape(128, HT * 512)).astype(f16)

    W2T = np.ascontiguousarray(W2.T)         # [3200, 3200] (in, out)
    w2PM = np.ascontiguousarray(
        W2T.reshape(HT, 128, N_MC, MCW).transpose(1, 2, 0, 3)
           .reshape(128, H * H // 128))
    if W2FP8:
        w2PM = (w2PM * W2SCALE).astype(ml_dtypes.float8_e4m3fn)
    else:
        w2PM = w2PM.astype(f16)

    W1T33 = np.zeros((33, H), np.float32)
    W1T33[:C] = W1.T
    W1T33[C] = b1
    W1T33 = W1T33.astype(f16)
    return packsF, b2R, w3PM, W1T33, w2PM


def _build_nc():
    import concourse.bacc as bacc
    import concourse.mybir as mybir
    from concourse import tile

    f32 = mybir.dt.float32
    f16 = mybir.dt.float16
    w2dt = mybir.dt.float8e4 if W2FP8 else f16
    prelu_scale = (1.0 / W2SCALE) if W2FP8 else 1.0

    nc = bacc.Bacc("TRN2", target_bir_lowering=False, debug=False, num_devices=NCORES)
    sm_d = nc.dram_tensor("sm_d", [128, HT], f32, kind="ExternalInput").ap()
    packs_d = nc.dram_tensor("packs_d", [128, PACKW], f16, kind="ExternalInput").ap()
    w3_d = nc.dram_tensor("w3_d", [128, HT * 512], f16, kind="ExternalInput").ap()
    dw_d = nc.dram_tensor("dw_d", [33, BL + H], f16, kind="ExternalInput").ap()
    w2_d = nc.dram_tensor("w2_d", [128, N_MC * W2CH], w2dt, kind="ExternalInput").ap()
    out_d = nc.dram_tensor("out_d", [128, SW], f32, kind="ExternalOutput").ap()

    Act = mybir.ActivationFunctionType
    Alu = mybir.AluOpType

    with tile.TileContext(nc) as tc:
        with tc.tile_pool(name="sb", bufs=1) as sb, \
             tc.tile_pool(name="wst", bufs=3) as wst, \
             tc.tile_pool(name="mlp", bufs=1) as mlp, \
             tc.tile_pool(name="ps", bufs=6, space="PSUM") as pspool, \
             tc.tile_pool(name="pb", bufs=2, space="PSUM") as pbpool:
            dw = mlp.tile([33, BL + H], f16)
            nc.sync.dma_start(out=dw[:], in_=dw_d[:])
            sm = sb.tile([128, HT], f32)
            nc.sync.dma_start(out=sm[:], in_=sm_d[:])
            w3sb = sb.tile([128, HT * 512], f16)
            packs = sb.tile([128, PACKW], f16)

            h1 = mlp.tile([128, H], f16)
            h2 = mlp.tile([128, H], f16)
            w_r = sb.tile([128, 512], f16)
            S0 = sb.tile([128, 128], f16)
            SBt = sb.tile([128, 512], f16)
            r_sb = sb.tile([128, SW], f32)
            q_sb = sb.tile([128, SW], f32)
            a_sb = sb.tile([128, SW], f16)
            o_sb = sb.tile([128, SW], f32)
            ones_t = sb.tile([1, 128], f16)
            nc.vector.memset(ones_t[:], 1.0)
            nc.vector.memset(S0[:], 0.0)
            nc.vector.memset(S0[32:33, :], 1.0)
            nc.vector.memset(o_sb[:, OK_:OK_ + 128], 0.0)

            # ---- MLP layer 1: h1 = prelu(W1^T d + b1) (b1 on ones-row 32) ----
            dT = dw[:, 0:BL]
            for m in range(HT):
                pst = pspool.tile([128, 128], f32, tag="ps", name="ps_t")
                nc.tensor.matmul(pst[:], dw[:, BL + m * 128:BL + (m + 1) * 128],
                                 dT, start=True, stop=True)
                mm = slice(m * 128, (m + 1) * 128)
                nc.scalar.activation(h1[:, mm], pst[:], Act.Prelu, alpha=0.1)

            # ---- MLP layer 2 + cost accumulating in one PSUM bank ----
            pcost = pbpool.tile([128, 512], f32, tag="pb", name="pcost")
            cost_first = [True]

            def cost_piece(mc, ki):
                k = mc * MC_W + ki
                for m in range(CT):
                    nc.tensor.matmul(pcost[:, m * 128:(m + 1) * 128],
                                     w3sb[:, k * 512 + m * 128:k * 512 + (m + 1) * 128],
                                     h2[:, k * 128:(k + 1) * 128],
                                     start=cost_first[0], stop=False,
                                     skip_group_check=True)
                    cost_first[0] = False

            def emit_cost(mc):
                for ki in range(MC_W):
                    cost_piece(mc, ki)

            for mc in range(N_MC):
                w2blk = wst.tile([128, W2CH], w2dt, name="w2blk")
                for s0, s1 in zip(W2SPLIT[:-1], W2SPLIT[1:]):
                    nc.sync.dma_start(out=w2blk[:, s0:s1],
                                      in_=w2_d[:, mc * W2CH + s0:mc * W2CH + s1])
                if mc < N_MC - 1:
                    nc.sync.dma_start(
                        out=w3sb[:, mc * MC_W * 512:(mc + 1) * MC_W * 512],
                        in_=w3_d[:, mc * MC_W * 512:(mc + 1) * MC_W * 512])
                if mc == N_MC - 2:
                    nc.sync.dma_start(
                        out=w3sb[:, (N_MC - 1) * MC_W * 512:],
                        in_=w3_d[:, (N_MC - 1) * MC_W * 512:])
                    nc.sync.dma_start(out=packs[:], in_=packs_d[:])
                if mc >= 1:
                    emit_cost(mc - 1)
                ps_list = [pspool.tile([128, 128], f32, tag="ps", name="ps_t")
                           for _ in range(MC_W)]
                last_mc = (mc == N_MC - 1)
                KSPLIT = 17 if last_mc else HT
                for k in range(KSPLIT):
                    for mi in range(MC_W):
                        nc.tensor.matmul(ps_list[mi][:],
                                         w2blk[:, k * MCW + mi * 128:
                                                  k * MCW + (mi + 1) * 128],
                                         h1[:, k * 128:(k + 1) * 128],
                                         start=(k == 0), stop=(k == HT - 1))
                if not last_mc:
                    for mi in range(MC_W):
                        m = mc * MC_W + mi
                        nc.scalar.activation(h2[:, m * 128:(m + 1) * 128],
                                             ps_list[mi][:], Act.Prelu,
                                             bias=sm[:, m:m + 1], alpha=0.1,
                                             scale=prelu_scale)
            # last chunk tail: mi-grouped third + pipelined cost
            mc = N_MC - 1
            for mi in range(MC_W):
                for k in range(17, HT):
                    nc.tensor.matmul(ps_list[mi][:],
                                     w2blk[:, k * MCW + mi * 128:
                                              k * MCW + (mi + 1) * 128],
                                     h1[:, k * 128:(k + 1) * 128],
                                     start=False, stop=(k == HT - 1))
                m = mc * MC_W + mi
                nc.scalar.activation(h2[:, m * 128:(m + 1) * 128], ps_list[mi][:],
                                     Act.Prelu, bias=sm[:, m:m + 1], alpha=0.1,
                                     scale=prelu_scale)
                if mi >= 1:
                    cost_piece(mc, mi - 1)
            cost_piece(mc, MC_W - 1)
            # b3 via rank-1 ones-row matmuls; closes the cost accumulation
            for m in range(CT):
                nc.tensor.matmul(pcost[:, m * 128:(m + 1) * 128],
                                 packs[0:1, PK_B3 + m * 128:PK_B3 + (m + 1) * 128],
                                 ones_t[0:1, :], start=False, stop=(m == CT - 1),
                                 skip_group_check=True)
            nc.scalar.activation(w_r[:], pcost[:], Act.Copy)

            # ---- ADMM in r/m form ----
            def chain(rhs_bot, S0t):
                """N-apply: returns (pV, pU0, pUb) PSUM tiles = N(a) blocks."""
                pV = pbpool.tile([128, 512], f32, tag="pb", name="pV")
                pU0 = pspool.tile([128, 128], f32, tag="ps", name="pU0")
                pUb = pbpool.tile([128, 512], f32, tag="pb", name="pUb")

                def rhs_of(t):
                    return S0t[:, :] if t == 0 else rhs_bot[:, (t - 1) * 128:t * 128]

                korder = [1, 2, 3, 4, 0]
                for m in range(4):
                    for ti, t in enumerate(korder):
                        nc.tensor.matmul(pV[:, m * 128:(m + 1) * 128],
                                         packs[:, PK_GR + (m * 5 + t) * 128:
                                                  PK_GR + (m * 5 + t + 1) * 128],
                                         rhs_of(t), start=(m == 0 and ti == 0),
                                         stop=(m == 3 and ti == 4),
                                         skip_group_check=True)
                for ti, t in enumerate(korder):
                    nc.tensor.matmul(pU0[0:30, :],
                                     packs[:, PK_MK + t * 30:PK_MK + (t + 1) * 30],
                                     rhs_of(t), start=(ti == 0), stop=(ti == 4))
                for m in range(4):
                    for ti, t in enumerate(korder):
                        nc.tensor.matmul(pUb[:, m * 128:(m + 1) * 128],
                                         packs[:, PK_MR + (m * 5 + t) * 128:
                                                  PK_MR + (m * 5 + t + 1) * 128],
                                         rhs_of(t), start=(m == 0 and ti == 0),
                                         stop=(m == 3 and ti == 4),
                                         skip_group_check=True)
                return pV, pU0, pUb

            def s_top_mm(rhs_r):
                pstS = pspool.tile([128, 128], f32, tag="ps", name="pS")
                for j in range(4):
                    nc.tensor.matmul(pstS[0:30, :],
                                     packs[:, PK_WMT + j * 30:PK_WMT + (j + 1) * 30],
                                     rhs_r[:, j * 128:(j + 1) * 128],
                                     start=(j == 0), stop=(j == 3))
                return pstS

            # r-pass (ones row active: packs fold -2c offsets)
            pstS = s_top_mm(w_r)
            nc.scalar.activation(S0[0:30, :], pstS[0:30, :], Act.Copy)
            pV, pU0, pUb = chain(w_r, S0)
            nc.vector.tensor_tensor(out=r_sb[:, 0:512], in0=w_r[:], in1=pV[:],
                                    op=Alu.subtract)
            nc.scalar.activation(a_sb[:, 0:512], r_sb[:, 0:512], Act.Abs)
            nc.scalar.activation(r_sb[0:30, OK_:OK_ + 128], pU0[0:30, :],
                                 Act.Copy, scale=-1.0)
            nc.scalar.activation(a_sb[0:30, OK_:OK_ + 128], pU0[0:30, :], Act.Abs)
            nc.scalar.activation(r_sb[:, OS_:], pUb[:], Act.Copy, scale=-1.0)
            nc.scalar.activation(a_sb[:, OS_:], pUb[:], Act.Abs)
            nc.vector.memset(S0[32:33, :], 0.0)   # ones row off for iterations

            # iterations 2..TOTAL
            for i in range(2, TOTAL + 1):
                pr = (i <= NPR)
                last = (i == TOTAL)
                plain = (not pr) and (not last)
                # S_bot = a_r + a_s
                nc.vector.tensor_tensor(out=SBt[:], in0=a_sb[:, 0:512],
                                        in1=a_sb[:, OS_:], op=Alu.add)
                # pre-adds riding the chain shadow
                if plain:
                    # t1 = q + r  (first plain iter reads q from last PR iter)
                    nc.vector.tensor_tensor(out=q_sb[:, 0:512], in0=q_sb[:, 0:512],
                                            in1=r_sb[:, 0:512], op=Alu.add)
                    nc.gpsimd.tensor_tensor(out=q_sb[0:30, OK_:OK_ + 128],
                                            in0=q_sb[0:30, OK_:OK_ + 128],
                                            in1=r_sb[0:30, OK_:OK_ + 128], op=Alu.add)
                    nc.vector.tensor_tensor(out=q_sb[:, OS_:], in0=q_sb[:, OS_:],
                                            in1=r_sb[:, OS_:], op=Alu.add)
                if last:
                    # t2 = r + a
                    nc.vector.tensor_tensor(out=q_sb[:, 0:512], in0=r_sb[:, 0:512],
                                            in1=a_sb[:, 0:512], op=Alu.add)
                    nc.gpsimd.tensor_tensor(out=q_sb[0:30, OK_:OK_ + 128],
                                            in0=r_sb[0:30, OK_:OK_ + 128],
                                            in1=a_sb[0:30, OK_:OK_ + 128], op=Alu.add)
                    nc.vector.tensor_tensor(out=q_sb[:, OS_:], in0=r_sb[:, OS_:],
                                            in1=a_sb[:, OS_:], op=Alu.add)
                pstS = s_top_mm(a_sb[:, 0:512])
                nc.vector.tensor_tensor(out=S0[0:30, :], in0=pstS[0:30, :],
                                        in1=a_sb[0:30, OK_:OK_ + 128], op=Alu.add)
                pV, pU0, pUb = chain(SBt, S0)
                if last:
                    # out2x = t2 - m ; host multiplies by 0.5
                    nc.vector.tensor_tensor(out=o_sb[:, 0:512], in0=q_sb[:, 0:512],
                                            in1=pV[:], op=Alu.subtract)
                    nc.sync.dma_start(out=out_d[:, 0:512], in_=o_sb[:, 0:512])
                    nc.vector.tensor_tensor(out=o_sb[0:30, OK_:OK_ + 128],
                                            in0=q_sb[0:30, OK_:OK_ + 128],
                                            in1=pU0[0:30, :], op=Alu.subtract)
                    nc.vector.tensor_tensor(out=o_sb[:, OS_:], in0=q_sb[:, OS_:],
                                            in1=pUb[:], op=Alu.subtract)
                    nc.sync.dma_start(out=out_d[:, OK_:], in_=o_sb[:, OK_:])
                elif pr:
                    # q' = r - m ; a' = |q'|
                    nc.vector.tensor_tensor(out=q_sb[:, 0:512], in0=r_sb[:, 0:512],
                                            in1=pV[:], op=Alu.subtract)
                    nc.scalar.activation(a_sb[:, 0:512], q_sb[:, 0:512], Act.Abs)
                    nc.vector.tensor_tensor(out=q_sb[0:30, OK_:OK_ + 128],
                                            in0=r_sb[0:30, OK_:OK_ + 128],
                                            in1=pU0[0:30, :], op=Alu.subtract)
                    nc.scalar.activation(a_sb[0:30, OK_:OK_ + 128],
                                         q_sb[0:30, OK_:OK_ + 128], Act.Abs)
                    nc.vector.tensor_tensor(out=q_sb[:, OS_:], in0=r_sb[:, OS_:],
                                            in1=pUb[:], op=Alu.subtract)
                    nc.scalar.activation(a_sb[:, OS_:], q_sb[:, OS_:], Act.Abs)
                else:
                    # plain: d = t1 - m (t1 in q_sb); a' = |d|/2; q' = d/2 unused
                    # for the last-but-one iter when NFIN == 2
                    nc.vector.tensor_tensor(out=q_sb[:, 0:512], in0=q_sb[:, 0:512],
                                            in1=pV[:], op=Alu.subtract)
                    nc.scalar.activation(a_sb[:, 0:512], q_sb[:, 0:512], Act.Abs,
                                         scale=0.5)
                    nc.vector.tensor_tensor(out=q_sb[0:30, OK_:OK_ + 128],
                                            in0=q_sb[0:30, OK_:OK_ + 128],
                                            in1=pU0[0:30, :], op=Alu.subtract)
                    nc.scalar.activation(a_sb[0:30, OK_:OK_ + 128],
                                         q_sb[0:30, OK_:OK_ + 128], Act.Abs,
                                         scale=0.5)
                    nc.vector.tensor_tensor(out=q_sb[:, OS_:], in0=q_sb[:, OS_:],
                                            in1=pUb[:], op=Alu.subtract)
                    nc.scalar.activation(a_sb[:, OS_:], q_sb[:, OS_:], Act.Abs,
                                         scale=0.5)

    nc.compile()
    return nc


def kernel(d, W1, b1, W2, b2, W3, b3, weights_mat, capacities):
    import ml_dtypes
    from concourse.bass_utils import run_bass_kernel_spmd

    d = np.asarray(d, np.float32)
    packsF, b2R, w3PM, W1T33, w2PM = _host_precompute(
        np.asarray(W1, np.float32), np.asarray(b1, np.float32),
        np.asarray(W2, np.float32), np.asarray(b2, np.float32),
        np.asarray(W3, np.float32), np.asarray(b3, np.float32),
        np.asarray(weights_mat, np.float32), np.asarray(capacities, np.float32))

    if "nc" not in _CACHE:
        _CACHE["nc"] = _build_nc()
    nc = _CACHE["nc"]

    in_maps = []
    for i in range(NCORES):
        dTc = np.zeros((33, BL), np.float16)
        dTc[:C] = d[i * BL:(i + 1) * BL].T.astype(np.float16)
        dTc[C] = 1.0
        dwc = np.ascontiguousarray(np.concatenate([dTc, W1T33], axis=1))
        in_maps.append({"sm_d": b2R, "packs_d": packsF,
                        "w3_d": w3PM, "dw_d": dwc, "w2_d": w2PM})

    trace = bool(int(os.environ.get("KNAP_TRACE", "0")))
    res = run_bass_kernel_spmd(nc, in_maps, core_ids=list(range(NCORES)),
                               trace=trace)
    if trace:
        _CACHE["exec_time_ns"] = res.exec_time_ns
        _CACHE["trace"] = res.instructions_and_trace

    out = np.empty((B, N2), np.float32)
    for i in range(NCORES):
        arr = 0.5 * res.results[i]["out_d"]                    # [128, 1152]
        xr = arr[:, 0:512].reshape(128, 4, 128).transpose(2, 1, 0).reshape(BL, 512)
        xk = arr[0:30, 512:640].T                              # [BL, 30]
        xs = arr[:, 640:1152].reshape(128, 4, 128).transpose(2, 1, 0).reshape(BL, 512)
        out[i * BL:(i + 1) * BL, 0:R] = xr[:, :R]
        out[i * BL:(i + 1) * BL, R:R + K] = xk
        out[i * BL:(i + 1) * BL, R + K:] = xs[:, :R]
    return out
